# revision 56
# baseline (speedup 1.0000x reference)
"""Trainium2 Bass kernel for nn_GATNodeScorer (GNN message passing).

Strategy (8 NeuronCores, node-partitioned, slot-aligned edge packing):
  - Host: sort nodes by in-degree; tile (core, round) gets 125 consecutive
    sorted nodes (+3 spare slots).  All 8 cores' tiles in round j share one
    chunk count K_j = max degree in the round, so the SPMD program is
    identical across cores and per-core work is balanced.  Edges are packed
    so that slot p of chunk k holds an edge whose DESTINATION is slot p:
    segment-sum collapses to a plain reduction over chunks and per-edge
    a_dst is a direct slot lookup -- no one-hot matmuls, no transposes.
  - Algebraic collapse of the network tail: there is no nonlinearity after
    GAT layer 1, so layer 2 only ever sees h2 through the linear maps
    C2 = [W2*Wo per head | W2@Asrc2 | W2@Adst2] (256x12).  Distributing C2
    through layer 1's per-head softmax aggregation, the layer-1 message
    carry shrinks from 256 columns to G = x1 @ C2-blocked (4 heads x 12)
    plus a_src1: 52 f32 per node.  Layer 2's carry is y2/a2src: 8 f32.
    Both GAT tables therefore fit 256-byte gather rows, dense2 disappears,
    and the final scatter produces scores directly.
  - Device, per core (SPMD, one NEFF):
      1. transposed projection hT = WpT @ xcT (Wp stationary, bias folded
         into the relu activation); u = h @ [W1G | W1@As1 | W1@Ad1]
         (56 cols, f32r); AllGather the u table (256B bf16 rows)
      2. relational layer x1aug = u + segsum(u[src]) + RW @ (rel_emb @
         d1aug) via four quarter-dma_gathers per tile on the 4 SWDGE
         queues + PE identity-accumulate; AllGather table-1 (G f32 +
         a_src f32 bitcast in 256B bf16 rows)
      3. GAT layer 1: per-tile quarter-gathers; alpha -> ex =
         max(exp(a), exp(0.2a)) (= exp(leakyrelu), 2 scaled ACT exps) ->
         [P,K,4,12] multiply + chunk tree; per-head normalize; + b1@C2
         -> layer-2 carries (y2/a2src f32 in 256B rows); AllGather
      4. GAT layer 2: per-tile quarter-gathers; softmax over y2 ->
         score = sum_h num_h/den_h + (b2@Wo + bo)
  - Padding gathers row 127, forced to payload=0 / a_src=-100 so ex ~ 0.

Self-contained: hardcodes all shapes; only needs numpy + the concourse repo
installed at /opt/trn_rl_repo.
"""

import sys

sys.path.insert(0, "/opt/trn_rl_repo")

import numpy as np
import ml_dtypes

import concourse.bass as bass
import concourse.bacc as bacc
import concourse.mybir as mybir
import concourse.tile as tile
from concourse.bass_utils import run_bass_kernel_spmd
from concourse.masks import make_identity

# ---- problem constants (hardcoded per contest rules) ----
N, E = 20000, 320000
IN_DIM, CODE_DIM, HIDDEN, HEADS, NREL = 896, 768, 256, 4, 5
CH = HIDDEN // HEADS
CODE_WEIGHT = 3.0
NEG_SLOPE = 0.2

NCORES = 8
P = 128
T = 20  # rounds (tiles per core)
NTILES = NCORES * T  # 160
NP = T * P  # 2560 padded nodes per core
NPAD = NTILES * P  # 20480
NPT = 125  # real nodes per tile (160*125 = 20000)
MROW = 127  # global row used for padding gathers (forced content)
KPROJ = IN_DIM // P  # 7

F32 = mybir.dt.float32
F32R = mybir.dt.float32r
BF16 = mybir.dt.bfloat16
I16 = mybir.dt.int16
NPBF = np.dtype(ml_dtypes.bfloat16)

NC2 = 12  # C2 columns: [y2(4) | a2src(4) | a2dst(4)]
NG = HEADS * NC2  # 48 f32: per-head x1 @ C2 carry
D1W = NG + 2 * HEADS  # 56 dense-1 output cols: [G | a1src | a1dst]
TW = 128  # gathered table row: 128 bf16 = 256B
AFO = 48  # f32 col of a_src within a table-1 row (bf16 cols 96..104)
AFO2 = 4  # f32 col of a_src within a table-2 row
UW = NG + 2 * HEADS  # 56: u = h @ d1aug row (bf16 cols 0..56 of a u-table row)
NRELP = 6

# ---------------------------------------------------------------------------
# host-side planning
# ---------------------------------------------------------------------------


def _build_plan(edge_index):
    src = edge_index[0].astype(np.int64)
    dst = edge_index[1].astype(np.int64)
    indeg = np.bincount(dst, minlength=N)
    order = np.argsort(-indeg, kind="stable")
    ranks = np.empty(N, np.int64)
    ranks[order] = np.arange(N)
    grp = ranks // NPT
    perm = (grp % NCORES) * NP + (grp // NCORES) * P + (ranks % NPT)

    K_rel = np.array(
        [
            int(indeg[order[j * NCORES * NPT : (j + 1) * NCORES * NPT]].max())
            for j in range(T)
        ],
        np.int64,
    )
    K_gat = K_rel + 1

    pd = perm[dst]
    order_e = np.argsort(pd, kind="stable")
    sd = pd[order_e]
    ps = perm[src][order_e].astype(np.int16)
    starts = np.r_[0, np.flatnonzero(np.diff(sd)) + 1]
    kk = np.arange(E, dtype=np.int64) - np.repeat(
        starts, np.diff(np.r_[starts, E])
    )
    ec = sd // NP
    erem = sd % NP
    ej = erem // P
    es = erem % P

    offs_rel = np.r_[0, np.cumsum(K_rel)]
    offs_gat = np.r_[0, np.cumsum(K_gat)]
    SR = 8 * int(K_rel.sum())
    SG = 8 * int(K_gat.sum())
    eidx_rel = np.full((NCORES, 128, SR), MROW, np.int16)
    eidx_gat = np.full((NCORES, 128, SG), MROW, np.int16)

    for j in range(T):
        KG = int(K_gat[j])
        m = ej == j
        A = np.full((NCORES, P, KG), MROW, np.int16)
        sidx = np.arange(NPT)
        for c in range(NCORES):
            A[c, :NPT, 0] = (c * NP + j * P + sidx).astype(np.int16)
        A[ec[m], es[m], kk[m] + 1] = ps[m]
        for c in range(NCORES):
            vg = np.ascontiguousarray(A[c].T).ravel()
            img = np.ascontiguousarray(vg.reshape(-1, 16).T)
            eidx_gat[c, :, 8 * offs_gat[j] : 8 * offs_gat[j + 1]] = np.tile(
                img, (8, 1)
            )
            vr = np.ascontiguousarray(A[c, :, 1:].T).ravel()
            imgr = np.ascontiguousarray(vr.reshape(-1, 16).T)
            eidx_rel[c, :, 8 * offs_rel[j] : 8 * offs_rel[j + 1]] = np.tile(
                imgr, (8, 1)
            )

    return dict(
        perm=perm,
        K_rel=tuple(int(k) for k in K_rel),
        K_gat=tuple(int(k) for k in K_gat),
        offs_rel=tuple(int(o) for o in offs_rel),
        offs_gat=tuple(int(o) for o in offs_gat),
        eidx_rel=eidx_rel,
        eidx_gat=eidx_gat,
    )


def _make_mrow():
    """Padding-target rows.  Row 0: zeros (u-table force).  Row 1: table-1
    force (a_src f32 at col AFO = -100).  Row 2: table-2 force (a_src f32
    at col AFO2 = -100) so exp(leakyrelu(alpha)) ~ 0 for padding edges."""
    rows = np.zeros((3, HIDDEN), NPBF)
    m100 = np.full(HEADS, -100.0, np.float32).view(np.uint8)
    rows.view(np.uint8)[1, 4 * AFO : 4 * AFO + 16] = m100
    rows.view(np.uint8)[2, 4 * AFO2 : 4 * AFO2 + 16] = m100
    return rows


def _asrc_mat(att):
    """[HEADS, CH] -> [HIDDEN, HEADS] block matrix so x @ A == (x*att).sum(-1)."""
    A = np.zeros((HIDDEN, HEADS), np.float32)
    for h in range(HEADS):
        A[h * CH : (h + 1) * CH, h] = att[h]
    return A


# ---------------------------------------------------------------------------
# bass program
# ---------------------------------------------------------------------------


def _tree_reduce(nc, src, acc, K, CW):
    """Sum K chunks of width CW from src ([P, K*CW]) into acc
    (f32 [P, ceil(K/2)*CW]); returns AP [P, CW] f32."""
    h = K // 2
    odd = K % 2
    if h == 0:
        nc.vector.tensor_copy(acc[:, :CW], src[:, :CW])
        return acc[:, :CW]
    nc.vector.tensor_add(acc[:, : h * CW], src[:, : h * CW], src[:, h * CW : 2 * h * CW])
    if odd:
        nc.vector.tensor_copy(
            acc[:, h * CW : (h + 1) * CW], src[:, 2 * h * CW : (2 * h + 1) * CW]
        )
        h += 1
    while h > 1:
        hh = h // 2
        odd = h % 2
        nc.vector.tensor_add(
            acc[:, : hh * CW], acc[:, : hh * CW], acc[:, hh * CW : 2 * hh * CW]
        )
        if odd:
            nc.vector.tensor_add(
                acc[:, :CW], acc[:, :CW], acc[:, 2 * hh * CW : (2 * hh + 1) * CW]
            )
        h = hh
    return acc[:, :CW]


def _build_bass(K_rel, K_gat, offs_rel, offs_gat, probe=None):
    probe = probe or {}
    reps = probe.get("reps", 1)
    Kmax = max(K_gat)
    KRmax = max(K_rel)
    SR = 8 * sum(K_rel)
    SG = 8 * sum(K_gat)
    nc = bacc.Bacc(
        "TRN2",
        target_bir_lowering=False,
        debug=False,
        num_devices=NCORES,
        num_swdge_queues=4,
    )

    # ---- external inputs ----
    xtt_in = nc.dram_tensor("xtt", [T, P, KPROJ * P], BF16, kind="ExternalInput")
    wp_in = nc.dram_tensor("wp", [KPROJ, P, HIDDEN], BF16, kind="ExternalInput")
    bp_in = nc.dram_tensor("bp_cols", [P, 2], F32, kind="ExternalInput")
    d1_in = nc.dram_tensor("d1aug", [2, P, D1W], F32, kind="ExternalInput")
    b1c2_in = nc.dram_tensor("b1c2_rep", [P, NC2], F32, kind="ExternalInput")
    rel_in = nc.dram_tensor("rel_d1", [NRELP, UW], F32, kind="ExternalInput")
    rwt_in = nc.dram_tensor("rwT", [NRELP, NP], F32, kind="ExternalInput")
    scb_in = nc.dram_tensor("sc_bias", [P, 1], F32, kind="ExternalInput")
    er_in = nc.dram_tensor("eidx_rel", [128, SR], I16, kind="ExternalInput")
    eg_in = nc.dram_tensor("eidx_gat", [128, SG], I16, kind="ExternalInput")
    mrow_in = nc.dram_tensor("mrow", [3, HIDDEN], BF16, kind="ExternalInput")

    score_out = nc.dram_tensor("score", [NP], F32, kind="ExternalOutput")

    with tile.TileContext(nc) as tc:
        with (
            tc.tile_pool(name="const", bufs=1) as cpool,
            tc.tile_pool(name="hres", bufs=1) as hpool,
            tc.tile_pool(name="lhsT", bufs=6) as lpool,
            tc.tile_pool(name="grel", bufs=6) as grpool,
            tc.tile_pool(name="gedge", bufs=6) as gepool,
            tc.tile_pool(name="mt", bufs=4) as mpool,
            tc.tile_pool(name="acc", bufs=2) as apool,
            tc.tile_pool(name="small", bufs=3) as spool,
            tc.tile_pool(name="ps", bufs=1, space="PSUM") as pspool,
            tc.tile_pool(name="dram", bufs=1, space="DRAM") as dpool,
        ):
            # ---- constants ----
            ident = cpool.tile([P, P], F32)
            make_identity(nc, ident[:])
            ident_bf = cpool.tile([P, P], BF16)
            nc.vector.tensor_copy(ident_bf[:], ident[:])

            wp_sb = cpool.tile([P, KPROJ * HIDDEN], BF16)
            for k in range(KPROJ):
                nc.sync.dma_start(
                    wp_sb[:, k * HIDDEN : (k + 1) * HIDDEN], wp_in[k, :, :]
                )
            bp_sb = cpool.tile([P, 2], F32)
            nc.sync.dma_start(bp_sb[:], bp_in[:, :])

            w_scr = cpool.tile([P, 2 * D1W], F32)
            d1_sb = cpool.tile([P, 2 * D1W], F32R)
            for k in range(2):
                nc.sync.dma_start(w_scr[:, k * D1W : (k + 1) * D1W], d1_in[k, :, :])
            nc.vector.tensor_copy(d1_sb[:], w_scr[:])

            b1c2_sb = cpool.tile([P, NC2], F32)
            nc.sync.dma_start(b1c2_sb[:], b1c2_in[:, :])
            rel_sb = cpool.tile([NRELP, UW], F32)
            nc.sync.dma_start(rel_sb[:], rel_in[:, :])
            rwt_sb = cpool.tile([NRELP, NP], F32)
            nc.sync.dma_start(rwt_sb[:], rwt_in[:, :])
            scb_sb = cpool.tile([P, 1], F32)
            nc.sync.dma_start(scb_sb[:], scb_in[:, :])
            er_sb = cpool.tile([128, SR], I16)
            nc.sync.dma_start(er_sb[:], er_in[:, :])
            eg_sb = cpool.tile([128, SG], I16)
            nc.sync.dma_start(eg_sb[:], eg_in[:, :])

            # resident u = h @ d1aug rows + per-tile a_dst columns (ping-pong
            # by rep parity so rep r+1 can start before rep r drains)
            u_alls = [hpool.tile([P, T * UW], F32, name=f"ua{i}") for i in range(2)]
            adst_alls = [
                hpool.tile([P, T * HEADS], F32, name=f"ad{i}") for i in range(2)
            ]

            # DRAM bounce buffers for collectives
            u_slabs = [dpool.tile([NP, TW], BF16, name=f"u_sl{r}") for r in range(reps)]
            t1_slabs = [dpool.tile([NP, TW], BF16, name=f"t1_sl{r}") for r in range(reps)]
            t2_slabs = [dpool.tile([NP, TW], BF16, name=f"t2_sl{r}") for r in range(reps)]
            u_fulls = [
                dpool.tile([NPAD, TW], BF16, addr_space="Shared", name=f"u_full{r}")
                for r in range(reps)
            ]
            t1_fulls = [
                dpool.tile([NPAD, TW], BF16, addr_space="Shared", name=f"t1_full{r}")
                for r in range(reps)
            ]
            t2_fulls = [
                dpool.tile([NPAD, TW], BF16, addr_space="Shared", name=f"t2_full{r}")
                for r in range(reps)
            ]

            def emit_gatherq(xa, full, idx_sb, off, K, W, t, nq):
                """One tile's gather as nq part-gathers on nq SWDGE queues."""
                cuts = [round(i * K / nq) for i in range(nq + 1)]
                for i in range(nq):
                    a, b = cuts[i], cuts[i + 1]
                    if b <= a:
                        continue
                    nc.gpsimd.dma_gather(
                        out_ap=xa[:, a * W : b * W].rearrange(
                            "p (k w) -> p k w", k=b - a
                        ),
                        in_ap=full[:, :],
                        idxs_ap=idx_sb[:, 8 * (off + a) : 8 * (off + b)],
                        num_idxs=(b - a) * 128,
                        num_idxs_reg=(b - a) * 128,
                        elem_size=W,
                        single_packet=False,
                        queue_num=(nq * t + i) % 4,
                    )

            def emit_rel(t, u_full):
                """x1aug(t) = u(t) + segsum(u[src]) + RW @ rel_d1; slab-1 rows."""
                K = K_rel[t]
                uch = grpool.tile([P, KRmax * TW], BF16, tag="grel")
                emit_gatherq(uch, u_full, er_sb, offs_rel[t], K, TW, t, 4)
                seg_ps = pspool.tile([P, UW], F32, tag="relps", bufs=4)
                nc.tensor.matmul(
                    out=seg_ps[:],
                    lhsT=rwt_sb[:, t * P : (t + 1) * P],
                    rhs=rel_sb[:],
                    start=True,
                    stop=False,
                )
                for k in range(K):
                    nc.tensor.matmul(
                        out=seg_ps[:],
                        lhsT=ident_bf[:],
                        rhs=uch[:, k * TW : k * TW + UW],
                        start=False,
                        stop=(k == K - 1),
                    )
                x1 = spool.tile([P, UW], F32, tag="x1")
                nc.vector.tensor_add(
                    x1[:], seg_ps[:], u_all[:, t * UW : (t + 1) * UW]
                )
                sl = spool.tile([P, TW], BF16, tag="sl")
                slf = sl[:].bitcast(F32)
                nc.vector.tensor_copy(slf[:, 0 : NG + HEADS], x1[:, 0 : NG + HEADS])
                nc.vector.tensor_copy(
                    adst_all[:, t * HEADS : (t + 1) * HEADS],
                    x1[:, NG + HEADS : UW],
                )
                if t == 0:
                    nc.scalar.dma_start(sl[MROW : MROW + 1, :], mrow_in[1:2, 0:TW])
                nc.scalar.dma_start(t1_slab[t * P : (t + 1) * P, :], sl[:])

            def emit_edge1(t, full):
                """GAT layer 1 for tile t -> layer-2 carries in slab-2."""
                K = K_gat[t]
                xa = gepool.tile([P, Kmax * TW], BF16, tag="gedge")
                emit_gatherq(xa, full, eg_sb, offs_gat[t], K, TW, t, 4)
                af = xa[:, : K * TW].bitcast(F32).rearrange("p (k w) -> p k w", k=K)
                alpha = spool.tile([P, Kmax * HEADS], F32, tag="alpha")
                nc.vector.tensor_tensor(
                    out=alpha[:, : K * HEADS].rearrange("p (k h) -> p k h", k=K),
                    in0=af[:, :, AFO : AFO + HEADS],
                    in1=adst_all[:, t * HEADS : (t + 1) * HEADS]
                    .unsqueeze(1)
                    .to_broadcast([P, K, HEADS]),
                    op=mybir.AluOpType.add,
                )
                e1 = spool.tile([P, Kmax * HEADS], BF16, tag="e1")
                nc.scalar.activation(
                    out=e1[:, : K * HEADS],
                    in_=alpha[:, : K * HEADS],
                    func=mybir.ActivationFunctionType.Exp,
                )
                e2 = spool.tile([P, Kmax * HEADS], BF16, tag="e2")
                nc.scalar.activation(
                    out=e2[:, : K * HEADS],
                    in_=alpha[:, : K * HEADS],
                    scale=NEG_SLOPE,
                    func=mybir.ActivationFunctionType.Exp,
                )
                ex = spool.tile([P, Kmax * HEADS], BF16, tag="ex")
                nc.vector.tensor_tensor(
                    out=ex[:, : K * HEADS],
                    in0=e1[:, : K * HEADS],
                    in1=e2[:, : K * HEADS],
                    op=mybir.AluOpType.max,
                )
                ex_v = ex[:, : K * HEADS].rearrange("p (k h) -> p k h", k=K)
                CW = NG + HEADS  # 52: [ex*G | ex]
                mt = mpool.tile([P, Kmax * CW], F32, tag="mt")
                mt_v = mt[:, : K * CW].rearrange("p (k w) -> p k w", k=K)
                nc.vector.tensor_tensor(
                    out=mt_v[:, :, 0:NG].rearrange("p k (h j) -> p k h j", h=HEADS),
                    in0=af[:, :, 0:NG].rearrange("p k (h j) -> p k h j", h=HEADS),
                    in1=ex_v.unsqueeze(-1).to_broadcast([P, K, HEADS, NC2]),
                    op=mybir.AluOpType.mult,
                )
                nc.vector.tensor_copy(mt_v[:, :, NG:CW], ex_v)
                acc = apool.tile([P, ((Kmax + 1) // 2) * CW], F32, tag="acc")
                tot = _tree_reduce(nc, mt[:, : K * CW], acc, K, CW)
                dinv = spool.tile([P, HEADS], F32, tag="dinv")
                nc.vector.reciprocal(dinv[:], tot[:, NG:CW])
                q = spool.tile([P, NG], F32, tag="q")
                nc.vector.tensor_tensor(
                    out=q[:].rearrange("p (h j) -> p h j", h=HEADS),
                    in0=tot[:, 0:NG].rearrange("p (h j) -> p h j", h=HEADS),
                    in1=dinv[:].unsqueeze(-1).to_broadcast([P, HEADS, NC2]),
                    op=mybir.AluOpType.mult,
                )
                # sum over the 4 layer-1 heads, then + b1@C2
                hs = spool.tile([P, 2 * NC2], F32, tag="hs")
                nc.vector.tensor_add(hs[:], q[:, 0 : 2 * NC2], q[:, 2 * NC2 : NG])
                vals = spool.tile([P, NC2], F32, tag="vals")
                nc.vector.tensor_add(vals[:], hs[:, 0:NC2], hs[:, NC2 : 2 * NC2])
                nc.vector.tensor_add(vals[:], vals[:], b1c2_sb[:])
                # layer-2 carries: [y2 f32 (cols 0:4) | a2src f32 (4:8)]
                sl = spool.tile([P, TW], BF16, tag="sl")
                slf = sl[:].bitcast(F32)
                nc.vector.tensor_copy(slf[:, 0 : 2 * HEADS], vals[:, 0 : 2 * HEADS])
                nc.vector.tensor_copy(
                    adst_all[:, t * HEADS : (t + 1) * HEADS],
                    vals[:, 2 * HEADS : 3 * HEADS],
                )
                if t == 0:
                    nc.scalar.dma_start(sl[MROW : MROW + 1, :], mrow_in[2:3, 0:TW])
                nc.scalar.dma_start(t2_slab[t * P : (t + 1) * P, :], sl[:])

            def emit_edge2(t, full):
                """GAT layer 2 + score for tile t."""
                K = K_gat[t]
                xa = gepool.tile([P, Kmax * TW], BF16, tag="gedge")
                emit_gatherq(xa, full, eg_sb, offs_gat[t], K, TW, t, 4)
                af = xa[:, : K * TW].bitcast(F32).rearrange("p (k w) -> p k w", k=K)
                alpha = spool.tile([P, Kmax * HEADS], F32, tag="alpha")
                nc.vector.tensor_tensor(
                    out=alpha[:, : K * HEADS].rearrange("p (k h) -> p k h", k=K),
                    in0=af[:, :, AFO2 : AFO2 + HEADS],
                    in1=adst_all[:, t * HEADS : (t + 1) * HEADS]
                    .unsqueeze(1)
                    .to_broadcast([P, K, HEADS]),
                    op=mybir.AluOpType.add,
                )
                e1 = spool.tile([P, Kmax * HEADS], BF16, tag="e1")
                nc.scalar.activation(
                    out=e1[:, : K * HEADS],
                    in_=alpha[:, : K * HEADS],
                    func=mybir.ActivationFunctionType.Exp,
                )
                e2 = spool.tile([P, Kmax * HEADS], BF16, tag="e2")
                nc.scalar.activation(
                    out=e2[:, : K * HEADS],
                    in_=alpha[:, : K * HEADS],
                    scale=NEG_SLOPE,
                    func=mybir.ActivationFunctionType.Exp,
                )
                ex = spool.tile([P, Kmax * HEADS], BF16, tag="ex")
                nc.vector.tensor_tensor(
                    out=ex[:, : K * HEADS],
                    in0=e1[:, : K * HEADS],
                    in1=e2[:, : K * HEADS],
                    op=mybir.AluOpType.max,
                )
                ex_v = ex[:, : K * HEADS].rearrange("p (k h) -> p k h", k=K)
                CW = 2 * HEADS  # 8: [ex*y2 | ex]
                mt = mpool.tile([P, Kmax * CW], F32, tag="mt2")
                mt_v = mt[:, : K * CW].rearrange("p (k w) -> p k w", k=K)
                nc.vector.tensor_tensor(
                    out=mt_v[:, :, 0:HEADS],
                    in0=af[:, :, 0:HEADS],
                    in1=ex_v,
                    op=mybir.AluOpType.mult,
                )
                nc.vector.tensor_copy(mt_v[:, :, HEADS:CW], ex_v)
                acc = apool.tile([P, ((Kmax + 1) // 2) * CW], F32, tag="acc2")
                tot = _tree_reduce(nc, mt[:, : K * CW], acc, K, CW)
                dinv = spool.tile([P, HEADS], F32, tag="dinv")
                nc.vector.reciprocal(dinv[:], tot[:, HEADS:CW])
                sch = spool.tile([P, HEADS], F32, tag="sch")
                nc.vector.tensor_mul(sch[:], tot[:, 0:HEADS], dinv[:])
                red = spool.tile([P, 1], F32, tag="red")
                nc.vector.tensor_reduce(
                    out=red[:],
                    in_=sch[:],
                    axis=mybir.AxisListType.X,
                    op=mybir.AluOpType.add,
                )
                sc = spool.tile([P, 1], F32, tag="sc")
                nc.vector.tensor_add(sc[:], red[:], scb_sb[:])
                nc.scalar.dma_start(score_out[t * P : (t + 1) * P], sc[:])

            def emit_ag(slab, full):
                if probe.get("no_collective"):
                    nc.sync.dma_start(full[0:NP, :], slab[:, :])
                else:
                    nc.gpsimd.collective_compute(
                        "AllGather",
                        mybir.AluOpType.bypass,
                        replica_groups=[list(range(NCORES))],
                        ins=[slab.opt()],
                        outs=[full.opt()],
                    )

            for rep in range(reps):
                u_full = u_fulls[rep]
                t1_full = t1_fulls[rep]
                t2_full = t2_fulls[rep]
                u_slab = u_slabs[rep]
                t1_slab = t1_slabs[rep]
                t2_slab = t2_slabs[rep]
                u_all = u_alls[rep % 2]
                adst_all = adst_alls[rep % 2]

                # ====== stage 1: transposed projection + u = h @ d1aug ======
                for t in range(T):
                    lx = lpool.tile([P, KPROJ * P], BF16, tag="lhsT")
                    nc.sync.dma_start(lx[:], xtt_in[t, :, :])
                    hfT = spool.tile([P, HIDDEN], F32, tag="hf")
                    for half in range(2):
                        hT_ps = pspool.tile([P, P], F32, tag="xps", bufs=4)
                        for k in range(KPROJ):
                            nc.tensor.matmul(
                                out=hT_ps[:],
                                lhsT=wp_sb[
                                    :, k * HIDDEN + half * P : k * HIDDEN + (half + 1) * P
                                ],
                                rhs=lx[:, k * P : (k + 1) * P],
                                start=(k == 0),
                                stop=(k == KPROJ - 1),
                            )
                        nc.scalar.activation(
                            out=hfT[:, half * P : (half + 1) * P],
                            in_=hT_ps[:],
                            bias=bp_sb[:, half : half + 1],
                            func=mybir.ActivationFunctionType.Relu,
                        )
                    hfr = lpool.tile([P, HIDDEN], F32R, tag="lhsTr")
                    nc.vector.tensor_copy(hfr[:], hfT[:])
                    u_ps = pspool.tile([P, UW], F32, tag="relps", bufs=4)
                    for half in range(2):
                        nc.tensor.matmul(
                            out=u_ps[:],
                            lhsT=hfr[:, half * P : (half + 1) * P],
                            rhs=d1_sb[:, half * D1W : (half + 1) * D1W],
                            start=(half == 0),
                            stop=(half == 1),
                        )
                    nc.vector.tensor_copy(u_all[:, t * UW : (t + 1) * UW], u_ps[:])
                    usl = spool.tile([P, TW], BF16, tag="hsl")
                    nc.vector.tensor_copy(usl[:, 0:UW], u_ps[:])
                    if t == 0:
                        nc.scalar.dma_start(usl[MROW : MROW + 1, :], mrow_in[0:1, 0:TW])
                    nc.scalar.dma_start(u_slab[t * P : (t + 1) * P, :], usl[:])

                if probe.get("stop_after") == "proj":
                    continue
                emit_ag(u_slab, u_full)

                # ====== stage 2: relational layer -> table-1 slabs ======
                for t in range(T):
                    emit_rel(t, u_full)
                if probe.get("stop_after") == "rel":
                    continue
                emit_ag(t1_slab, t1_full)

                # ====== stage 3: GAT layer 1 (emits layer-2 carries) ======
                for t in range(T):
                    emit_edge1(t, t1_full)
                if probe.get("stop_after") == "gat1":
                    continue
                emit_ag(t2_slab, t2_full)

                # ====== stage 4: GAT layer 2 + score ======
                for t in range(T):
                    emit_edge2(t, t2_full)

    nc.compile()
    return nc


# ---------------------------------------------------------------------------
# entry point
# ---------------------------------------------------------------------------

_CACHE = {}


def prepare(inputs, plan, probe=None):
    """Build (in_maps, nc, perm) from the full input dict + plan."""
    x = np.asarray(inputs["x"], np.float32)
    edge_index = np.asarray(inputs["edge_index"], np.int32)
    edge_type = np.asarray(inputs["edge_type"], np.int32)
    edge_weight = np.asarray(inputs["edge_weight"], np.float32)
    rel_emb = np.asarray(inputs["rel_emb"], np.float32)
    Wp = np.asarray(inputs["Wp"], np.float32)
    bp = np.asarray(inputs["bp"], np.float32)
    W1 = np.asarray(inputs["W1"], np.float32)
    W2 = np.asarray(inputs["W2"], np.float32)
    att_src1 = np.asarray(inputs["att_src1"], np.float32)
    att_dst1 = np.asarray(inputs["att_dst1"], np.float32)
    att_src2 = np.asarray(inputs["att_src2"], np.float32)
    att_dst2 = np.asarray(inputs["att_dst2"], np.float32)
    b1 = np.asarray(inputs["b1"], np.float32)
    b2 = np.asarray(inputs["b2"], np.float32)
    Wo = np.asarray(inputs["Wo"], np.float32)
    bo = np.asarray(inputs["bo"], np.float32)

    perm = plan["perm"]

    # ---- per-core dense inputs ----
    xr = np.concatenate([x[:, CODE_DIM:], CODE_WEIGHT * x[:, :CODE_DIM]], axis=1)
    xpad = np.zeros((NPAD, IN_DIM), np.float32)
    xpad[perm] = xr
    # [C, T, P(feat-within-chunk), KPROJ*P(node)] so one DMA loads a tile's
    # whole lhsT set
    xtt = (
        xpad.reshape(NCORES, T, P, KPROJ, P)
        .transpose(0, 1, 4, 3, 2)
        .reshape(NCORES, T, P, KPROJ * P)
        .astype(NPBF)
    )

    # ---- algebraic collapse of the network tail ----
    # C2 = [per-head W2*Wo | W2@Asrc2 | W2@Adst2]  (256 x 12)
    W2y = np.stack(
        [W2[:, h * CH : (h + 1) * CH] @ Wo[h * CH : (h + 1) * CH, 0] for h in range(HEADS)],
        axis=1,
    )
    C2 = np.concatenate([W2y, W2 @ _asrc_mat(att_src2), W2 @ _asrc_mat(att_dst2)], axis=1)
    # G carry: W1G[:, h*12+j] = W1[:, hC:(h+1)C] @ C2[hC:(h+1)C, j]
    W1G = np.zeros((HIDDEN, NG), np.float32)
    for h in range(HEADS):
        W1G[:, h * NC2 : (h + 1) * NC2] = (
            W1[:, h * CH : (h + 1) * CH] @ C2[h * CH : (h + 1) * CH, :]
        )
    d1aug = np.concatenate(
        [W1G, W1 @ _asrc_mat(att_src1), W1 @ _asrc_mat(att_dst1)], axis=1
    )
    b1c2 = b1 @ C2  # [12]
    sc_bias = float(b2 @ Wo[:, 0] + bo[0])

    # ---- per-node relation histogram: RW[n, r] = sum of w_e over in-edges ----
    RW = np.zeros((NPAD, NRELP), np.float32)
    np.add.at(RW, (perm[edge_index[1].astype(np.int64)], edge_type), edge_weight)

    key = (plan["K_rel"], plan["K_gat"], tuple(sorted((probe or {}).items())))
    if key not in _CACHE:
        _CACHE[key] = _build_bass(
            plan["K_rel"], plan["K_gat"], plan["offs_rel"], plan["offs_gat"], probe
        )
    nc = _CACHE[key]

    common = dict(
        wp=np.ascontiguousarray(Wp.reshape(KPROJ, P, HIDDEN)).astype(NPBF),
        bp_cols=np.ascontiguousarray(bp.reshape(2, P).T),
        d1aug=np.ascontiguousarray(
            np.stack([d1aug[:P], d1aug[P:]], axis=0)
        ),
        b1c2_rep=np.ascontiguousarray(np.broadcast_to(b1c2, (P, NC2))),
        rel_d1=np.ascontiguousarray(
            np.concatenate(
                [rel_emb, np.zeros((NRELP - NREL, HIDDEN), np.float32)]
            )
            @ d1aug
        ),
        sc_bias=np.full((P, 1), sc_bias, np.float32),
        mrow=_make_mrow(),
    )
    in_maps = []
    for c in range(NCORES):
        in_maps.append(
            dict(
                common,
                xtt=xtt[c],
                rwT=np.ascontiguousarray(RW[c * NP : (c + 1) * NP, :].T),
                eidx_rel=plan["eidx_rel"][c],
                eidx_gat=plan["eidx_gat"][c],
            )
        )
    return in_maps, nc, perm


def kernel(x, edge_index, **rest):
    inputs = dict(rest, x=x, edge_index=edge_index)
    edge_index = np.asarray(edge_index, np.int32)
    plan = _build_plan(edge_index)
    in_maps, nc, perm = prepare(inputs, plan)

    import os

    trace = bool(os.environ.get("GAT_TRACE"))
    res = run_bass_kernel_spmd(
        nc, in_maps, core_ids=list(range(NCORES)), trace=trace
    )
    global _LAST_RESULT
    _LAST_RESULT = res
    scores_pad = np.concatenate([r["score"] for r in res.results])
    return scores_pad[perm].astype(np.float32)


_LAST_RESULT = None


# revision 57
# speedup vs baseline: 1.1095x; 1.1095x over previous
"""Trainium2 Bass kernel for nn_GATNodeScorer (GNN message passing).

Strategy (8 NeuronCores, node-partitioned, slot-aligned edge packing):
  - Host: sort nodes by in-degree; tile (core, round) gets 125 consecutive
    sorted nodes (+3 spare slots).  All 8 cores' tiles in round j share one
    chunk count K_j = max degree in the round, so the SPMD program is
    identical across cores and per-core work is balanced.  Edges are packed
    so that slot p of chunk k holds an edge whose DESTINATION is slot p:
    segment-sum collapses to a plain reduction over chunks and per-edge
    a_dst is a direct slot lookup -- no one-hot matmuls, no transposes.
  - Algebraic collapse of the network tail: there is no nonlinearity after
    GAT layer 1, so layer 2 only ever sees h2 through the linear maps
    C2 = [W2*Wo per head | W2@Asrc2 | W2@Adst2] (256x12).  Distributing C2
    through layer 1's per-head softmax aggregation, the layer-1 message
    carry shrinks from 256 columns to G = x1 @ C2-blocked (4 heads x 12)
    plus a_src1: 52 f32 per node.  Layer 2's carry is y2/a2src: 8 f32.
    Both GAT tables therefore fit 256-byte gather rows, dense2 disappears,
    and the final scatter produces scores directly.
  - Device, per core (SPMD, one NEFF):
      1. transposed projection hT = WpT @ xcT (Wp stationary, bias folded
         into the relu activation); u = h @ [W1G | W1@As1 | W1@Ad1]
         (56 cols, f32r); AllGather the u table (256B bf16 rows)
      2. relational layer x1aug = u + segsum(u[src]) + RW @ (rel_emb @
         d1aug) via four quarter-dma_gathers per tile on the 4 SWDGE
         queues + PE identity-accumulate; AllGather table-1 (G f32 +
         a_src f32 bitcast in 256B bf16 rows)
      3. GAT layer 1: per-tile quarter-gathers; alpha -> ex =
         max(exp(a), exp(0.2a)) (= exp(leakyrelu), 2 scaled ACT exps) ->
         [P,K,4,12] multiply + chunk tree; per-head normalize; + b1@C2
         -> layer-2 carries (y2/a2src f32 in 256B rows); AllGather
      4. GAT layer 2: per-tile quarter-gathers; softmax over y2 ->
         score = sum_h num_h/den_h + (b2@Wo + bo)
  - Padding gathers row 127, forced to payload=0 / a_src=-100 so ex ~ 0.

Self-contained: hardcodes all shapes; only needs numpy + the concourse repo
installed at /opt/trn_rl_repo.
"""

import sys

sys.path.insert(0, "/opt/trn_rl_repo")

import numpy as np
import ml_dtypes

import concourse.bass as bass
import concourse.bacc as bacc
import concourse.mybir as mybir
import concourse.tile as tile
from concourse.bass_utils import run_bass_kernel_spmd
from concourse.masks import make_identity

# ---- problem constants (hardcoded per contest rules) ----
N, E = 20000, 320000
IN_DIM, CODE_DIM, HIDDEN, HEADS, NREL = 896, 768, 256, 4, 5
CH = HIDDEN // HEADS
CODE_WEIGHT = 3.0
NEG_SLOPE = 0.2

NCORES = 8
P = 128
T = 20  # rounds (tiles per core)
NTILES = NCORES * T  # 160
NP = T * P  # 2560 padded nodes per core
NPAD = NTILES * P  # 20480
NPT = 125  # real nodes per tile (160*125 = 20000)
MROW = 127  # global row used for padding gathers (forced content)
KPROJ = IN_DIM // P  # 7

F32 = mybir.dt.float32
F32R = mybir.dt.float32r
BF16 = mybir.dt.bfloat16
I16 = mybir.dt.int16
NPBF = np.dtype(ml_dtypes.bfloat16)

NC2 = 12  # C2 columns: [y2(4) | a2src(4) | a2dst(4)]
NG = HEADS * NC2  # 48 f32: per-head x1 @ C2 carry
D1W = NG + 2 * HEADS  # 56 dense-1 output cols: [G | a1src | a1dst]
TW = 128  # gathered table row: 128 bf16 = 256B
AFO = 48  # f32 col of a_src within a table-1 row (bf16 cols 96..104)
AFO2 = 4  # f32 col of a_src within a table-2 row
UW = NG + 2 * HEADS  # 56: u = h @ d1aug row (bf16 cols 0..56 of a u-table row)
NRELP = 6

# ---------------------------------------------------------------------------
# host-side planning
# ---------------------------------------------------------------------------


def _build_plan(edge_index):
    src = edge_index[0].astype(np.int64)
    dst = edge_index[1].astype(np.int64)
    indeg = np.bincount(dst, minlength=N)
    order = np.argsort(-indeg, kind="stable")
    ranks = np.empty(N, np.int64)
    ranks[order] = np.arange(N)
    grp = ranks // NPT
    perm = (grp % NCORES) * NP + (grp // NCORES) * P + (ranks % NPT)

    K_rel = np.array(
        [
            int(indeg[order[j * NCORES * NPT : (j + 1) * NCORES * NPT]].max())
            for j in range(T)
        ],
        np.int64,
    )
    K_gat = K_rel + 1

    pd = perm[dst]
    order_e = np.argsort(pd, kind="stable")
    sd = pd[order_e]
    ps = perm[src][order_e].astype(np.int16)
    starts = np.r_[0, np.flatnonzero(np.diff(sd)) + 1]
    kk = np.arange(E, dtype=np.int64) - np.repeat(
        starts, np.diff(np.r_[starts, E])
    )
    ec = sd // NP
    erem = sd % NP
    ej = erem // P
    es = erem % P

    offs_rel = np.r_[0, np.cumsum(K_rel)]
    offs_gat = np.r_[0, np.cumsum(K_gat)]
    SR = 8 * int(K_rel.sum())
    SG = 8 * int(K_gat.sum())
    eidx_rel = np.full((NCORES, 128, SR), MROW, np.int16)
    eidx_gat = np.full((NCORES, 128, SG), MROW, np.int16)

    for j in range(T):
        KG = int(K_gat[j])
        m = ej == j
        A = np.full((NCORES, P, KG), MROW, np.int16)
        sidx = np.arange(NPT)
        for c in range(NCORES):
            A[c, :NPT, 0] = (c * NP + j * P + sidx).astype(np.int16)
        A[ec[m], es[m], kk[m] + 1] = ps[m]
        for c in range(NCORES):
            vg = np.ascontiguousarray(A[c].T).ravel()
            img = np.ascontiguousarray(vg.reshape(-1, 16).T)
            eidx_gat[c, :, 8 * offs_gat[j] : 8 * offs_gat[j + 1]] = np.tile(
                img, (8, 1)
            )
            vr = np.ascontiguousarray(A[c, :, 1:].T).ravel()
            imgr = np.ascontiguousarray(vr.reshape(-1, 16).T)
            eidx_rel[c, :, 8 * offs_rel[j] : 8 * offs_rel[j + 1]] = np.tile(
                imgr, (8, 1)
            )

    return dict(
        perm=perm,
        K_rel=tuple(int(k) for k in K_rel),
        K_gat=tuple(int(k) for k in K_gat),
        offs_rel=tuple(int(o) for o in offs_rel),
        offs_gat=tuple(int(o) for o in offs_gat),
        eidx_rel=eidx_rel,
        eidx_gat=eidx_gat,
    )


def _make_mrow():
    """Padding-target rows.  Row 0: zeros (u-table force).  Row 1: table-1
    force (a_src f32 at col AFO = -100).  Row 2: table-2 force (a_src f32
    at col AFO2 = -100) so exp(leakyrelu(alpha)) ~ 0 for padding edges."""
    rows = np.zeros((3, HIDDEN), NPBF)
    m100 = np.full(HEADS, -100.0, np.float32).view(np.uint8)
    rows.view(np.uint8)[1, 4 * AFO : 4 * AFO + 16] = m100
    rows.view(np.uint8)[2, 4 * AFO2 : 4 * AFO2 + 16] = m100
    return rows


def _asrc_mat(att):
    """[HEADS, CH] -> [HIDDEN, HEADS] block matrix so x @ A == (x*att).sum(-1)."""
    A = np.zeros((HIDDEN, HEADS), np.float32)
    for h in range(HEADS):
        A[h * CH : (h + 1) * CH, h] = att[h]
    return A


# ---------------------------------------------------------------------------
# bass program
# ---------------------------------------------------------------------------


def _tree_reduce(nc, src, acc, K, CW):
    """Sum K chunks of width CW from src ([P, K*CW]) into acc
    (f32 [P, ceil(K/2)*CW]); returns AP [P, CW] f32."""
    h = K // 2
    odd = K % 2
    if h == 0:
        nc.vector.tensor_copy(acc[:, :CW], src[:, :CW])
        return acc[:, :CW]
    nc.vector.tensor_add(acc[:, : h * CW], src[:, : h * CW], src[:, h * CW : 2 * h * CW])
    if odd:
        nc.vector.tensor_copy(
            acc[:, h * CW : (h + 1) * CW], src[:, 2 * h * CW : (2 * h + 1) * CW]
        )
        h += 1
    while h > 1:
        hh = h // 2
        odd = h % 2
        nc.vector.tensor_add(
            acc[:, : hh * CW], acc[:, : hh * CW], acc[:, hh * CW : 2 * hh * CW]
        )
        if odd:
            nc.vector.tensor_add(
                acc[:, :CW], acc[:, :CW], acc[:, 2 * hh * CW : (2 * hh + 1) * CW]
            )
        h = hh
    return acc[:, :CW]


def _build_bass(K_rel, K_gat, offs_rel, offs_gat, probe=None):
    probe = probe or {}
    reps = probe.get("reps", 1)
    Kmax = max(K_gat)
    KRmax = max(K_rel)
    SR = 8 * sum(K_rel)
    SG = 8 * sum(K_gat)
    nc = bacc.Bacc(
        "TRN2",
        target_bir_lowering=False,
        debug=False,
        num_devices=NCORES,
        num_swdge_queues=4,
    )

    # ---- external inputs ----
    xtt_in = nc.dram_tensor("xtt", [T, P, KPROJ * P], BF16, kind="ExternalInput")
    wp_in = nc.dram_tensor("wp", [KPROJ, P, HIDDEN], BF16, kind="ExternalInput")
    bp_in = nc.dram_tensor("bp_cols", [P, 2], F32, kind="ExternalInput")
    d1_in = nc.dram_tensor("d1aug", [2, P, D1W], F32, kind="ExternalInput")
    b1c2_in = nc.dram_tensor("b1c2_rep", [P, NC2], F32, kind="ExternalInput")
    rel_in = nc.dram_tensor("rel_d1", [NRELP, UW], F32, kind="ExternalInput")
    rwt_in = nc.dram_tensor("rwT", [NRELP, NP], F32, kind="ExternalInput")
    scb_in = nc.dram_tensor("sc_bias", [P, 1], F32, kind="ExternalInput")
    er_in = nc.dram_tensor("eidx_rel", [128, SR], I16, kind="ExternalInput")
    eg_in = nc.dram_tensor("eidx_gat", [128, SG], I16, kind="ExternalInput")
    mrow_in = nc.dram_tensor("mrow", [3, HIDDEN], BF16, kind="ExternalInput")

    score_out = nc.dram_tensor("score", [NP], F32, kind="ExternalOutput")

    with tile.TileContext(nc) as tc:
        with (
            tc.tile_pool(name="const", bufs=1) as cpool,
            tc.tile_pool(name="hres", bufs=1) as hpool,
            tc.tile_pool(name="lhsT", bufs=6) as lpool,
            tc.tile_pool(name="grel", bufs=6) as grpool,
            tc.tile_pool(name="gedge", bufs=6) as gepool,
            tc.tile_pool(name="mt", bufs=4) as mpool,
            tc.tile_pool(name="acc", bufs=2) as apool,
            tc.tile_pool(name="small", bufs=3) as spool,
            tc.tile_pool(name="ps", bufs=1, space="PSUM") as pspool,
            tc.tile_pool(name="dram", bufs=1, space="DRAM") as dpool,
        ):
            # ---- constants ----
            ident = cpool.tile([P, P], F32)
            make_identity(nc, ident[:])
            ident_bf = cpool.tile([P, P], BF16)
            nc.vector.tensor_copy(ident_bf[:], ident[:])

            wp_sb = cpool.tile([P, KPROJ * HIDDEN], BF16)
            for k in range(KPROJ):
                nc.sync.dma_start(
                    wp_sb[:, k * HIDDEN : (k + 1) * HIDDEN], wp_in[k, :, :]
                )
            bp_sb = cpool.tile([P, 2], F32)
            nc.sync.dma_start(bp_sb[:], bp_in[:, :])

            w_scr = cpool.tile([P, 2 * D1W], F32)
            d1_sb = cpool.tile([P, 2 * D1W], F32R)
            for k in range(2):
                nc.sync.dma_start(w_scr[:, k * D1W : (k + 1) * D1W], d1_in[k, :, :])
            nc.vector.tensor_copy(d1_sb[:], w_scr[:])

            b1c2_sb = cpool.tile([P, NC2], F32)
            nc.sync.dma_start(b1c2_sb[:], b1c2_in[:, :])
            rel_sb = cpool.tile([NRELP, UW], F32)
            nc.sync.dma_start(rel_sb[:], rel_in[:, :])
            rwt_sb = cpool.tile([NRELP, NP], F32)
            nc.sync.dma_start(rwt_sb[:], rwt_in[:, :])
            scb_sb = cpool.tile([P, 1], F32)
            nc.sync.dma_start(scb_sb[:], scb_in[:, :])
            er_sb = cpool.tile([128, SR], I16)
            nc.sync.dma_start(er_sb[:], er_in[:, :])
            eg_sb = cpool.tile([128, SG], I16)
            nc.sync.dma_start(eg_sb[:], eg_in[:, :])

            # resident u = h @ d1aug rows + per-tile a_dst columns (ping-pong
            # by rep parity so rep r+1 can start before rep r drains)
            u_alls = [hpool.tile([P, T * UW], F32, name=f"ua{i}") for i in range(2)]
            adst_alls = [
                hpool.tile([P, T * HEADS], F32, name=f"ad{i}") for i in range(2)
            ]

            # DRAM bounce buffers for collectives
            u_slabs = [dpool.tile([NP, TW], BF16, name=f"u_sl{r}") for r in range(reps)]
            t1_slabs = [dpool.tile([NP, TW], BF16, name=f"t1_sl{r}") for r in range(reps)]
            t2_slabs = [dpool.tile([NP, TW], BF16, name=f"t2_sl{r}") for r in range(reps)]
            u_fulls = [
                dpool.tile([NPAD, TW], BF16, addr_space="Shared", name=f"u_full{r}")
                for r in range(reps)
            ]
            t1_fulls = [
                dpool.tile([NPAD, TW], BF16, addr_space="Shared", name=f"t1_full{r}")
                for r in range(reps)
            ]
            t2_fulls = [
                dpool.tile([NPAD, TW], BF16, addr_space="Shared", name=f"t2_full{r}")
                for r in range(reps)
            ]

            def emit_gatherq(xa, full, idx_sb, off, K, W, t, nq):
                """One tile's gather as nq part-gathers on nq SWDGE queues."""
                cuts = [round(i * K / nq) for i in range(nq + 1)]
                for i in range(nq):
                    a, b = cuts[i], cuts[i + 1]
                    if b <= a:
                        continue
                    nc.gpsimd.dma_gather(
                        out_ap=xa[:, a * W : b * W].rearrange(
                            "p (k w) -> p k w", k=b - a
                        ),
                        in_ap=full[:, :],
                        idxs_ap=idx_sb[:, 8 * (off + a) : 8 * (off + b)],
                        num_idxs=(b - a) * 128,
                        num_idxs_reg=(b - a) * 128,
                        elem_size=W,
                        single_packet=False,
                        queue_num=(nq * t + i) % 4,
                    )

            def emit_rel(t, u_full):
                """x1aug(t) = u(t) + segsum(u[src]) + RW @ rel_d1; slab-1 rows."""
                K = K_rel[t]
                uch = grpool.tile([P, KRmax * TW], BF16, tag="grel")
                emit_gatherq(uch, u_full, er_sb, offs_rel[t], K, TW, t, 4)
                seg_ps = pspool.tile([P, UW], F32, tag="relps", bufs=2)
                nc.tensor.matmul(
                    out=seg_ps[:],
                    lhsT=rwt_sb[:, t * P : (t + 1) * P],
                    rhs=rel_sb[:],
                    start=True,
                    stop=False,
                )
                for k in range(K):
                    nc.tensor.matmul(
                        out=seg_ps[:],
                        lhsT=ident_bf[:],
                        rhs=uch[:, k * TW : k * TW + UW],
                        start=False,
                        stop=(k == K - 1),
                    )
                x1 = spool.tile([P, UW], F32, tag="x1")
                nc.vector.tensor_add(
                    x1[:], seg_ps[:], u_all[:, t * UW : (t + 1) * UW]
                )
                sl = spool.tile([P, TW], BF16, tag="sl")
                slf = sl[:].bitcast(F32)
                nc.vector.tensor_copy(slf[:, 0 : NG + HEADS], x1[:, 0 : NG + HEADS])
                nc.vector.tensor_copy(
                    adst_all[:, t * HEADS : (t + 1) * HEADS],
                    x1[:, NG + HEADS : UW],
                )
                if t == 0:
                    nc.scalar.dma_start(sl[MROW : MROW + 1, :], mrow_in[1:2, 0:TW])
                nc.scalar.dma_start(t1_slab[t * P : (t + 1) * P, :], sl[:])

            def emit_edge1(t, full):
                """GAT layer 1 for tile t -> layer-2 carries in slab-2."""
                K = K_gat[t]
                xa = gepool.tile([P, Kmax * TW], BF16, tag="gedge")
                emit_gatherq(xa, full, eg_sb, offs_gat[t], K, TW, t, 4)
                af = xa[:, : K * TW].bitcast(F32).rearrange("p (k w) -> p k w", k=K)
                alpha = spool.tile([P, Kmax * HEADS], F32, tag="alpha")
                nc.vector.tensor_tensor(
                    out=alpha[:, : K * HEADS].rearrange("p (k h) -> p k h", k=K),
                    in0=af[:, :, AFO : AFO + HEADS],
                    in1=adst_all[:, t * HEADS : (t + 1) * HEADS]
                    .unsqueeze(1)
                    .to_broadcast([P, K, HEADS]),
                    op=mybir.AluOpType.add,
                )
                e1 = spool.tile([P, Kmax * HEADS], BF16, tag="e1")
                nc.scalar.activation(
                    out=e1[:, : K * HEADS],
                    in_=alpha[:, : K * HEADS],
                    func=mybir.ActivationFunctionType.Exp,
                )
                e2 = spool.tile([P, Kmax * HEADS], BF16, tag="e2")
                nc.scalar.activation(
                    out=e2[:, : K * HEADS],
                    in_=alpha[:, : K * HEADS],
                    scale=NEG_SLOPE,
                    func=mybir.ActivationFunctionType.Exp,
                )
                ex = spool.tile([P, Kmax * HEADS], BF16, tag="ex")
                nc.vector.tensor_tensor(
                    out=ex[:, : K * HEADS],
                    in0=e1[:, : K * HEADS],
                    in1=e2[:, : K * HEADS],
                    op=mybir.AluOpType.max,
                )
                ex_v = ex[:, : K * HEADS].rearrange("p (k h) -> p k h", k=K)
                CW = NG + HEADS  # 52: [ex*G | ex]
                mt = mpool.tile([P, Kmax * CW], F32, tag="mt")
                mt_v = mt[:, : K * CW].rearrange("p (k w) -> p k w", k=K)
                nc.vector.tensor_tensor(
                    out=mt_v[:, :, 0:NG].rearrange("p k (h j) -> p k h j", h=HEADS),
                    in0=af[:, :, 0:NG].rearrange("p k (h j) -> p k h j", h=HEADS),
                    in1=ex_v.unsqueeze(-1).to_broadcast([P, K, HEADS, NC2]),
                    op=mybir.AluOpType.mult,
                )
                nc.vector.tensor_copy(mt_v[:, :, NG:CW], ex_v)
                acc = apool.tile([P, ((Kmax + 1) // 2) * CW], F32, tag="acc")
                tot = _tree_reduce(nc, mt[:, : K * CW], acc, K, CW)
                dinv = spool.tile([P, HEADS], F32, tag="dinv")
                nc.vector.reciprocal(dinv[:], tot[:, NG:CW])
                q = spool.tile([P, NG], F32, tag="q")
                nc.vector.tensor_tensor(
                    out=q[:].rearrange("p (h j) -> p h j", h=HEADS),
                    in0=tot[:, 0:NG].rearrange("p (h j) -> p h j", h=HEADS),
                    in1=dinv[:].unsqueeze(-1).to_broadcast([P, HEADS, NC2]),
                    op=mybir.AluOpType.mult,
                )
                # sum over the 4 layer-1 heads, then + b1@C2
                hs = spool.tile([P, 2 * NC2], F32, tag="hs")
                nc.vector.tensor_add(hs[:], q[:, 0 : 2 * NC2], q[:, 2 * NC2 : NG])
                vals = spool.tile([P, NC2], F32, tag="vals")
                nc.vector.tensor_add(vals[:], hs[:, 0:NC2], hs[:, NC2 : 2 * NC2])
                nc.vector.tensor_add(vals[:], vals[:], b1c2_sb[:])
                # layer-2 carries: [y2 f32 (cols 0:4) | a2src f32 (4:8)]
                sl = spool.tile([P, TW], BF16, tag="sl")
                slf = sl[:].bitcast(F32)
                nc.vector.tensor_copy(slf[:, 0 : 2 * HEADS], vals[:, 0 : 2 * HEADS])
                nc.vector.tensor_copy(
                    adst_all[:, t * HEADS : (t + 1) * HEADS],
                    vals[:, 2 * HEADS : 3 * HEADS],
                )
                if t == 0:
                    nc.scalar.dma_start(sl[MROW : MROW + 1, :], mrow_in[2:3, 0:TW])
                nc.scalar.dma_start(t2_slab[t * P : (t + 1) * P, :], sl[:])

            def emit_edge2(t, full):
                """GAT layer 2 + score for tile t."""
                K = K_gat[t]
                xa = gepool.tile([P, Kmax * TW], BF16, tag="gedge")
                emit_gatherq(xa, full, eg_sb, offs_gat[t], K, TW, t, 4)
                af = xa[:, : K * TW].bitcast(F32).rearrange("p (k w) -> p k w", k=K)
                alpha = spool.tile([P, Kmax * HEADS], F32, tag="alpha")
                nc.vector.tensor_tensor(
                    out=alpha[:, : K * HEADS].rearrange("p (k h) -> p k h", k=K),
                    in0=af[:, :, AFO2 : AFO2 + HEADS],
                    in1=adst_all[:, t * HEADS : (t + 1) * HEADS]
                    .unsqueeze(1)
                    .to_broadcast([P, K, HEADS]),
                    op=mybir.AluOpType.add,
                )
                e1 = spool.tile([P, Kmax * HEADS], BF16, tag="e1")
                nc.scalar.activation(
                    out=e1[:, : K * HEADS],
                    in_=alpha[:, : K * HEADS],
                    func=mybir.ActivationFunctionType.Exp,
                )
                e2 = spool.tile([P, Kmax * HEADS], BF16, tag="e2")
                nc.scalar.activation(
                    out=e2[:, : K * HEADS],
                    in_=alpha[:, : K * HEADS],
                    scale=NEG_SLOPE,
                    func=mybir.ActivationFunctionType.Exp,
                )
                ex = spool.tile([P, Kmax * HEADS], BF16, tag="ex")
                nc.vector.tensor_tensor(
                    out=ex[:, : K * HEADS],
                    in0=e1[:, : K * HEADS],
                    in1=e2[:, : K * HEADS],
                    op=mybir.AluOpType.max,
                )
                ex_v = ex[:, : K * HEADS].rearrange("p (k h) -> p k h", k=K)
                CW = 2 * HEADS  # 8: [ex*y2 | ex]
                mt = mpool.tile([P, Kmax * CW], F32, tag="mt2")
                mt_v = mt[:, : K * CW].rearrange("p (k w) -> p k w", k=K)
                nc.vector.tensor_tensor(
                    out=mt_v[:, :, 0:HEADS],
                    in0=af[:, :, 0:HEADS],
                    in1=ex_v,
                    op=mybir.AluOpType.mult,
                )
                nc.vector.tensor_copy(mt_v[:, :, HEADS:CW], ex_v)
                acc = apool.tile([P, ((Kmax + 1) // 2) * CW], F32, tag="acc2")
                tot = _tree_reduce(nc, mt[:, : K * CW], acc, K, CW)
                dinv = spool.tile([P, HEADS], F32, tag="dinv")
                nc.vector.reciprocal(dinv[:], tot[:, HEADS:CW])
                sch = spool.tile([P, HEADS], F32, tag="sch")
                nc.vector.tensor_mul(sch[:], tot[:, 0:HEADS], dinv[:])
                red = spool.tile([P, 1], F32, tag="red")
                nc.vector.tensor_reduce(
                    out=red[:],
                    in_=sch[:],
                    axis=mybir.AxisListType.X,
                    op=mybir.AluOpType.add,
                )
                sc = spool.tile([P, 1], F32, tag="sc")
                nc.vector.tensor_add(sc[:], red[:], scb_sb[:])
                nc.scalar.dma_start(score_out[t * P : (t + 1) * P], sc[:])

            def emit_ag(slab, full):
                if probe.get("no_collective"):
                    nc.sync.dma_start(full[0:NP, :], slab[:, :])
                else:
                    nc.gpsimd.collective_compute(
                        "AllGather",
                        mybir.AluOpType.bypass,
                        replica_groups=[list(range(NCORES))],
                        ins=[slab.opt()],
                        outs=[full.opt()],
                    )

            for rep in range(reps):
                u_full = u_fulls[rep]
                t1_full = t1_fulls[rep]
                t2_full = t2_fulls[rep]
                u_slab = u_slabs[rep]
                t1_slab = t1_slabs[rep]
                t2_slab = t2_slabs[rep]
                u_all = u_alls[rep % 2]
                adst_all = adst_alls[rep % 2]

                # ====== stage 1: transposed projection + u = h @ d1aug ======
                for t in range(T):
                    lx = lpool.tile([P, KPROJ * P], BF16, tag="lhsT")
                    nc.sync.dma_start(lx[:], xtt_in[t, :, :])
                    hfT = spool.tile([P, HIDDEN], F32, tag="hf")
                    for half in range(2):
                        hT_ps = pspool.tile([P, P], F32, tag="xps", bufs=2)
                        for k in range(KPROJ):
                            nc.tensor.matmul(
                                out=hT_ps[:],
                                lhsT=wp_sb[
                                    :, k * HIDDEN + half * P : k * HIDDEN + (half + 1) * P
                                ],
                                rhs=lx[:, k * P : (k + 1) * P],
                                start=(k == 0),
                                stop=(k == KPROJ - 1),
                            )
                        nc.scalar.activation(
                            out=hfT[:, half * P : (half + 1) * P],
                            in_=hT_ps[:],
                            bias=bp_sb[:, half : half + 1],
                            func=mybir.ActivationFunctionType.Relu,
                        )
                    hfr = lpool.tile([P, HIDDEN], F32R, tag="lhsTr")
                    nc.vector.tensor_copy(hfr[:], hfT[:])
                    u_ps = pspool.tile([P, UW], F32, tag="relps", bufs=2)
                    for half in range(2):
                        nc.tensor.matmul(
                            out=u_ps[:],
                            lhsT=hfr[:, half * P : (half + 1) * P],
                            rhs=d1_sb[:, half * D1W : (half + 1) * D1W],
                            start=(half == 0),
                            stop=(half == 1),
                        )
                    nc.vector.tensor_copy(u_all[:, t * UW : (t + 1) * UW], u_ps[:])
                    usl = spool.tile([P, TW], BF16, tag="hsl")
                    nc.vector.tensor_copy(usl[:, 0:UW], u_ps[:])
                    if t == 0:
                        nc.scalar.dma_start(usl[MROW : MROW + 1, :], mrow_in[0:1, 0:TW])
                    nc.scalar.dma_start(u_slab[t * P : (t + 1) * P, :], usl[:])

                if probe.get("stop_after") == "proj":
                    continue
                emit_ag(u_slab, u_full)

                # ====== stage 2: relational layer -> table-1 slabs ======
                for t in range(T):
                    emit_rel(t, u_full)
                if probe.get("stop_after") == "rel":
                    continue
                emit_ag(t1_slab, t1_full)

                # ====== stage 3: GAT layer 1 (emits layer-2 carries) ======
                for t in range(T):
                    emit_edge1(t, t1_full)
                if probe.get("stop_after") == "gat1":
                    continue
                emit_ag(t2_slab, t2_full)

                # ====== stage 4: GAT layer 2 + score ======
                for t in range(T):
                    emit_edge2(t, t2_full)

    nc.compile()
    return nc


# ---------------------------------------------------------------------------
# entry point
# ---------------------------------------------------------------------------

_CACHE = {}


def prepare(inputs, plan, probe=None):
    """Build (in_maps, nc, perm) from the full input dict + plan."""
    x = np.asarray(inputs["x"], np.float32)
    edge_index = np.asarray(inputs["edge_index"], np.int32)
    edge_type = np.asarray(inputs["edge_type"], np.int32)
    edge_weight = np.asarray(inputs["edge_weight"], np.float32)
    rel_emb = np.asarray(inputs["rel_emb"], np.float32)
    Wp = np.asarray(inputs["Wp"], np.float32)
    bp = np.asarray(inputs["bp"], np.float32)
    W1 = np.asarray(inputs["W1"], np.float32)
    W2 = np.asarray(inputs["W2"], np.float32)
    att_src1 = np.asarray(inputs["att_src1"], np.float32)
    att_dst1 = np.asarray(inputs["att_dst1"], np.float32)
    att_src2 = np.asarray(inputs["att_src2"], np.float32)
    att_dst2 = np.asarray(inputs["att_dst2"], np.float32)
    b1 = np.asarray(inputs["b1"], np.float32)
    b2 = np.asarray(inputs["b2"], np.float32)
    Wo = np.asarray(inputs["Wo"], np.float32)
    bo = np.asarray(inputs["bo"], np.float32)

    perm = plan["perm"]

    # ---- per-core dense inputs ----
    xr = np.concatenate([x[:, CODE_DIM:], CODE_WEIGHT * x[:, :CODE_DIM]], axis=1)
    xpad = np.zeros((NPAD, IN_DIM), np.float32)
    xpad[perm] = xr
    # [C, T, P(feat-within-chunk), KPROJ*P(node)] so one DMA loads a tile's
    # whole lhsT set
    xtt = (
        xpad.reshape(NCORES, T, P, KPROJ, P)
        .transpose(0, 1, 4, 3, 2)
        .reshape(NCORES, T, P, KPROJ * P)
        .astype(NPBF)
    )

    # ---- algebraic collapse of the network tail ----
    # C2 = [per-head W2*Wo | W2@Asrc2 | W2@Adst2]  (256 x 12)
    W2y = np.stack(
        [W2[:, h * CH : (h + 1) * CH] @ Wo[h * CH : (h + 1) * CH, 0] for h in range(HEADS)],
        axis=1,
    )
    C2 = np.concatenate([W2y, W2 @ _asrc_mat(att_src2), W2 @ _asrc_mat(att_dst2)], axis=1)
    # G carry: W1G[:, h*12+j] = W1[:, hC:(h+1)C] @ C2[hC:(h+1)C, j]
    W1G = np.zeros((HIDDEN, NG), np.float32)
    for h in range(HEADS):
        W1G[:, h * NC2 : (h + 1) * NC2] = (
            W1[:, h * CH : (h + 1) * CH] @ C2[h * CH : (h + 1) * CH, :]
        )
    d1aug = np.concatenate(
        [W1G, W1 @ _asrc_mat(att_src1), W1 @ _asrc_mat(att_dst1)], axis=1
    )
    b1c2 = b1 @ C2  # [12]
    sc_bias = float(b2 @ Wo[:, 0] + bo[0])

    # ---- per-node relation histogram: RW[n, r] = sum of w_e over in-edges ----
    RW = np.zeros((NPAD, NRELP), np.float32)
    np.add.at(RW, (perm[edge_index[1].astype(np.int64)], edge_type), edge_weight)

    key = (plan["K_rel"], plan["K_gat"], tuple(sorted((probe or {}).items())))
    if key not in _CACHE:
        _CACHE[key] = _build_bass(
            plan["K_rel"], plan["K_gat"], plan["offs_rel"], plan["offs_gat"], probe
        )
    nc = _CACHE[key]

    common = dict(
        wp=np.ascontiguousarray(Wp.reshape(KPROJ, P, HIDDEN)).astype(NPBF),
        bp_cols=np.ascontiguousarray(bp.reshape(2, P).T),
        d1aug=np.ascontiguousarray(
            np.stack([d1aug[:P], d1aug[P:]], axis=0)
        ),
        b1c2_rep=np.ascontiguousarray(np.broadcast_to(b1c2, (P, NC2))),
        rel_d1=np.ascontiguousarray(
            np.concatenate(
                [rel_emb, np.zeros((NRELP - NREL, HIDDEN), np.float32)]
            )
            @ d1aug
        ),
        sc_bias=np.full((P, 1), sc_bias, np.float32),
        mrow=_make_mrow(),
    )
    in_maps = []
    for c in range(NCORES):
        in_maps.append(
            dict(
                common,
                xtt=xtt[c],
                rwT=np.ascontiguousarray(RW[c * NP : (c + 1) * NP, :].T),
                eidx_rel=plan["eidx_rel"][c],
                eidx_gat=plan["eidx_gat"][c],
            )
        )
    return in_maps, nc, perm


def kernel(x, edge_index, **rest):
    inputs = dict(rest, x=x, edge_index=edge_index)
    edge_index = np.asarray(edge_index, np.int32)
    plan = _build_plan(edge_index)
    in_maps, nc, perm = prepare(inputs, plan)

    import os

    trace = bool(os.environ.get("GAT_TRACE"))
    res = run_bass_kernel_spmd(
        nc, in_maps, core_ids=list(range(NCORES)), trace=trace
    )
    global _LAST_RESULT
    _LAST_RESULT = res
    scores_pad = np.concatenate([r["score"] for r in res.results])
    return scores_pad[perm].astype(np.float32)


_LAST_RESULT = None
